# revision 53
# baseline (speedup 1.0000x reference)
"""Trainium2 Bass kernel for nn_DetectionLoss (2-class detection loss).

Computes, over B=2^24 rows of logits [B,2] and labels [B]:
  ce    = mean(-log_softmax(outputs)[label])
  pred  = argmax(outputs, axis=1)
  confusion counts TP/TN/FP/FN from (label, pred)
  CS    = M[pred, label] with M = [[0,1],[0,0]]  -> mean(CS) = FN/B
  loss  = ce + coeff(TP,TN,FP,FN) * mean(CS)

Device math (2 classes): with d = x1 - x0 and h = label - 0.5 (labels
are staged host-side in their +-0.5 encoding as bf16, which is exact
for a binary label and halves their HBM footprint vs int32):
  u       = d*h                  # sign-folded logit margin
  ce_row  = softplus(-2u) = log(1 + exp(-2u))
  pred    = (d > 0)
  correct = (u > 0)              # prediction == label
Counts follow from three linear sums (n1 = sum(h) + B/2, p1 = sum(pred),
TP + TN = sum(correct) = C):
  TP = (C + p1 + n1 - B) / 2, TN = C - TP, FP = p1 - TP, FN = n1 - TP.

Engine split per chunk, balanced against the ~6.2 us the two input
DMAs of a 2048-row chunk take:
  DVE: d = x1 - x0;  u = d*h;  pred = (d>0) as fp8;
       correct = (u>0) as fp8 on the first half of columns
  ACT: t = exp(-2u); ln(1+t) with accum -> CE partial;
       Sign(u) on the second half of columns with accum
  PE : sum(h) via bf16 ones-matmuls; sum(pred)/sum(correct) via fp8
       DoubleRow ones-matmuls (2 input columns paired per output
       column - exact for 0/1 sums), accumulated in PSUM with two
       alternating banks per quantity
The tiny per-core partials are combined on the host; count arithmetic
is exact (half-integers in fp32 at every stage).

Sharding: data-parallel over the batch dim across 8 NeuronCores.
"""

import numpy as np

import concourse.bass as bass
import concourse.mybir as mybir
import concourse.tile as tile
from concourse.bass_utils import run_bass_kernel_spmd

N_CORES = 8
P = 128
LAMBD = 0.5
MMN = 512  # matmul rhs free-dim tile (one PSUM bank of fp32)

# fp8 DoubleRow matmul: pairs adjacent rhs columns, output is half-width.
PERF_MODE = mybir.MatmulPerfMode.DoubleRow

_cache = {}

_MAX_WAITS = 1  # this walrus build rejects >1 embedded sync-wait per instruction


def _split_multiwaits(nc):
    """Walrus in this container can't encode instructions with multiple
    sync waits; hoist all but the last into standalone EventSemaphore
    waits on the same engine immediately before the instruction."""
    n = [0]

    def fix_block(blk):
        new_insts = []
        for ins in blk.instructions:
            si = ins.sync_info
            if si is not None and si.on_wait and len(si.on_wait) > _MAX_WAITS:
                waits = list(si.on_wait)
                for w in waits[: -_MAX_WAITS]:
                    n[0] += 1
                    ev = mybir.InstEventSemaphore(
                        name=f"I-waitsplit-{n[0]}",
                        ins=[],
                        outs=[],
                        sync_info=mybir.SyncInfo(on_wait=[w], on_update=[]),
                    )
                    ev.engine = ins.engine
                    new_insts.append(ev)
                si.on_wait = waits[-_MAX_WAITS:]
            new_insts.append(ins)
        blk.instructions = new_insts

    for fn in nc.m.functions:
        for blk in fn.blocks:
            fix_block(blk)


def _chunk_plan(rpp: int):
    """Rows-per-partition per chunk. Small chunks at both ends shorten the
    pipeline fill (first compute can't start before chunk 0 lands) and the
    tail (last chunk's compute latency after the final DMA byte). The
    first two chunks must be >= 512 so the PSUM start slabs span their
    banks' full written width (see the matmul start= logic)."""
    if rpp == 16384:
        plan = (
            [1024, 1024, 1536]
            + [2048] * 5
            + [1024, 512, 512, 256, 256]
        )
    else:
        # small test sizes: 1024-row chunks
        assert rpp % 1024 == 0 and rpp >= 2048
        plan = [1024] * (rpp // 1024)
    assert sum(plan) == rpp and all(f % 256 == 0 for f in plan)
    assert plan[0] >= 1024 and plan[1] >= 1024
    return plan


def _eact_split(F: int) -> int:
    """Columns [0, fd) of `correct` go to DVE+PE; [fd, F) to ACT Sign.
    F <= 512 stays fully on DVE so the PSUM start slabs are full-width."""
    if F <= 1024:
        return F
    return max(1024, (5 * F // 8 // 256) * 256)


def _build(rows_per_core: int):
    """Build the per-core Bass module. All cores run the same program on
    their own shard (pure data parallel, no collectives)."""
    key = rows_per_core
    if key in _cache:
        return _cache[key]

    assert rows_per_core % P == 0
    rpp = rows_per_core // P  # rows per partition
    plan = _chunk_plan(rpp)
    nch = len(plan)
    fmax = max(plan)

    nc = bass.Bass(trn_type="TRN2")
    dtf = mybir.dt.float32
    dtb = mybir.dt.bfloat16
    dt8 = mybir.dt.float8e4
    Op = mybir.AluOpType
    Act = mybir.ActivationFunctionType

    x = nc.dram_tensor("x", [P, 2 * rpp], dtf, kind="ExternalInput")
    lab = nc.dram_tensor("lab", [P, rpp], dtb, kind="ExternalInput")
    # accumulator columns: [ce quarters | sign(u) upper part] x nch
    acc = nc.dram_tensor("acc", [P, 5 * nch], dtf, kind="ExternalOutput")
    # PE-reduced count partials: [sum(h) 2x512 | sum(pred) 2x256 |
    # sum(correct lower) 2x256]
    acc_h = nc.dram_tensor("acc_h", [1, 6 * MMN], dtf,
                           kind="ExternalOutput")

    slabs = lambda F: (F + MMN - 1) // MMN
    slabs2 = lambda F: (F + 2 * MMN - 1) // (2 * MMN)
    tot_h = sum(slabs(F) for F in plan)
    tot_p = sum(slabs2(F) for F in plan)
    tot_e = sum(slabs2(_eact_split(F)) for F in plan)

    with tile.TileContext(nc) as tc:
        with (
            tc.tile_pool(name="io", bufs=5) as io_pool,
            tc.tile_pool(name="mid", bufs=4) as mid,
            tc.tile_pool(name="junk", bufs=2) as junk,
            tc.tile_pool(name="singles", bufs=1) as singles,
            tc.tile_pool(name="ps", bufs=1, space="PSUM") as psp,
        ):
            ones = singles.tile([P, 1], dtb)
            nc.vector.memset(ones, 1.0)
            ones8 = singles.tile([P, 32], dt8)
            nc.vector.memset(ones8, 1.0)
            ones8_2 = ones8.rearrange("p (two m) -> p two m", two=2)
            st_ce = singles.tile([P, 4 * nch], dtf)
            st_sg = singles.tile([P, nch], dtf)
            nc.gpsimd.memset(st_ce, 0.0)
            nc.gpsimd.memset(st_sg, 0.0)
            ps_h = [
                psp.tile([1, MMN], dtf, tag=f"ps_h{i}", name=f"ps_h{i}")
                for i in range(2)
            ]
            ps_p = [
                psp.tile([16, MMN], dtf, tag=f"ps_p{i}", name=f"ps_p{i}")
                for i in range(2)
            ]
            ps_e = [
                psp.tile([16, MMN], dtf, tag=f"ps_e{i}", name=f"ps_e{i}")
                for i in range(2)
            ]

            r0 = 0
            ks_h = ks_p = ks_e = 0
            for c, F in enumerate(plan):
                r1 = r0 + F
                fd = _eact_split(F)
                xt_full = io_pool.tile([P, 2 * fmax], dtf, tag="xt")
                xt = xt_full[:, : 2 * F]
                nc.sync.dma_start(out=xt, in_=x[:, 2 * r0 : 2 * r1])
                xp = xt.rearrange("p (f two) -> p f two", two=2)
                lt_full = io_pool.tile([P, fmax], dtb, tag="lt")
                lv = lt_full[:, :F]
                nc.sync.dma_start(out=lv, in_=lab[:, r0:r1])

                # d = x1 - x0 and u = d*h, in column halves so the ACT
                # chain (which consumes u) can start ~half a chunk earlier
                d_full = mid.tile([P, fmax], dtb, tag="d")
                d = d_full[:, :F]
                u_full = mid.tile([P, fmax], dtb, tag="u")
                u = u_full[:, :F]
                fh0 = (F // 2 // 256) * 256 if F >= 512 else F
                for hs in (
                    (slice(0, fh0), slice(fh0, F)) if fh0 < F else (slice(0, F),)
                ):
                    nc.vector.tensor_sub(
                        out=d[:, hs], in0=xp[:, hs, 1], in1=xp[:, hs, 0]
                    )
                    nc.vector.tensor_mul(
                        out=u[:, hs], in0=d[:, hs], in1=lv[:, hs]
                    )
                # pred = (d > 0) as fp8 0/1
                jp_full = mid.tile([P, fmax], dt8, tag="jp")
                jp = jp_full[:, :F]
                nc.vector.tensor_scalar(
                    out=jp, in0=d, scalar1=0.0, scalar2=None, op0=Op.is_gt
                )
                # correct = (u > 0) as fp8 0/1 on columns [0, fd)
                je_full = mid.tile([P, fmax], dt8, tag="je")
                je = je_full[:, :fd]
                nc.vector.tensor_scalar(
                    out=je, in0=u[:, :fd], scalar1=0.0, scalar2=None,
                    op0=Op.is_gt
                )

                # CE partial on ACT: t = exp(-2u); ln(1+t), accum.
                # Done in two column halves so the ACT stream advances in
                # smaller steps (earlier cross-engine progress signals).
                t_full = mid.tile([P, fmax], dtb, tag="t")
                ja_full = junk.tile([P, fmax], dtb, tag="ja")
                nq = 4 if F >= 2048 else (2 if F >= 512 else 1)
                qw = (F // nq // 256) * 256
                cuts = [i * qw for i in range(nq)] + [F]
                for hi in range(nq):
                    hs = slice(cuts[hi], cuts[hi + 1])
                    th = t_full[:, hs]
                    nc.scalar.activation(
                        out=th, in_=u[:, hs], func=Act.Exp, scale=-2.0
                    )
                    jah = ja_full[:, hs]
                    nc.scalar.activation(
                        out=jah, in_=th, func=Act.Ln, bias=1.0, scale=1.0,
                        accum_out=st_ce[:, 4 * c + hi : 4 * c + hi + 1],
                    )
                # sum(sign(u)) over columns [fd, F) on ACT
                if fd < F:
                    js_full = junk.tile([P, fmax], dtb, tag="js")
                    js = js_full[:, : F - fd]
                    nc.scalar.activation(
                        out=js, in_=u[:, fd:], func=Act.Sign,
                        accum_out=st_sg[:, c : c + 1],
                    )

                # count sums on PE; banks alternate per 512-slab.
                for k in range(slabs(F)):
                    sl = slice(k * MMN, min((k + 1) * MMN, F))
                    w = sl.stop - sl.start
                    nc.tensor.matmul(
                        ps_h[ks_h % 2][:, :w], ones, lv[:, sl],
                        start=ks_h < 2, stop=ks_h >= tot_h - 2,
                    )
                    ks_h += 1
                for k in range(slabs2(F)):
                    sl = slice(k * 2 * MMN, min((k + 1) * 2 * MMN, F))
                    w = sl.stop - sl.start
                    jp2 = jp[:, sl].rearrange("p (n two) -> p two n", two=2)
                    nc.tensor.matmul(
                        ps_p[ks_p % 2][:, : w // 2], ones8_2, jp2,
                        start=ks_p < 2, stop=ks_p >= tot_p - 2,
                        perf_mode=PERF_MODE,
                    )
                    ks_p += 1
                for k in range(slabs2(fd)):
                    sl = slice(k * 2 * MMN, min((k + 1) * 2 * MMN, fd))
                    w = sl.stop - sl.start
                    je2 = je[:, sl].rearrange("p (n two) -> p two n", two=2)
                    nc.tensor.matmul(
                        ps_e[ks_e % 2][:, : w // 2], ones8_2, je2,
                        start=ks_e < 2, stop=ks_e >= tot_e - 2,
                        perf_mode=PERF_MODE,
                    )
                    ks_e += 1
                r0 = r1

            nc.sync.dma_start(out=acc[:, : 4 * nch], in_=st_ce)
            nc.sync.dma_start(out=acc[:, 4 * nch :], in_=st_sg)
            cnt_sb = singles.tile([1, 6 * MMN], dtf)
            off = 0
            for ps in ps_h + ps_p + ps_e:
                wps = ps.shape[1]
                nc.vector.tensor_copy(
                    out=cnt_sb[:, off : off + wps], in_=ps[0:1, :]
                )
                off += wps
            nc.sync.dma_start(out=acc_h[:], in_=cnt_sb)

    _cache[key] = (nc, nch, plan)
    return nc, nch, plan


def _combine(
    acc: np.ndarray, acc_h: np.ndarray, nch: int, plan, B: int
) -> np.ndarray:
    """Host-side scalar epilogue.

    acc: [n_cores, P, 2*nch] f32: [ce | sign(u) upper-column part].
    acc_h: [n_cores, 1, 2*MMN+4*256] f32 PE-reduced [sum(h) | sum(pred) |
    sum(correct lower)]. Counts are exact (half-)integers in fp32."""
    n_cores = acc.shape[0]
    a = acc.astype(np.float64)
    CE = a[:, :, : 4 * nch].sum()
    S_u = a[:, :, 4 * nch :].sum()
    ah = acc_h.astype(np.float64).reshape(n_cores, -1)
    H1 = ah[:, : 2 * MMN].sum()
    p1 = ah[:, 2 * MMN : 4 * MMN].sum()
    C_low = ah[:, 4 * MMN :].sum()
    n1 = H1 + B / 2.0  # labels == 1
    # rows covered by the ACT Sign path
    n_sign = n_cores * P * sum(F - _eact_split(F) for F in plan)
    C = C_low + (S_u + n_sign) / 2.0
    TP = (C + p1 + n1 - B) / 2.0
    TN = C - TP
    FP = p1 - TP
    FN = n1 - TP

    ce = CE / B
    mean_cs = FN / B
    nonzero = (TP > 0) and (TN > 0) and (FP > 0) and (FN > 0)
    ratio = (TP / max(TP + FN, 1.0)) * (FP / max(FP + TN, 1.0))
    if nonzero:
        coeff = -LAMBD * np.log(np.sqrt(max(ratio, 1e-30)))
    else:
        coeff = LAMBD
    return np.array(ce + coeff * mean_cs, dtype=np.float32)


def _stage_labels_bf16(labels: np.ndarray) -> np.ndarray:
    """Encode binary labels as bf16 h = label - 0.5 (+-0.5), exactly.
    0.5 -> 0x3F00, -0.5 -> 0xBF00 (as uint16 bit patterns)."""
    lab = labels.astype(bool)
    return np.where(lab, np.uint16(0x3F00), np.uint16(0xBF00))


def run(outputs: np.ndarray, labels: np.ndarray):
    """Run on 8 cores; returns (loss, BassKernelResults)."""
    outputs = np.asarray(outputs)
    labels = np.asarray(labels)
    B = outputs.shape[0]
    assert outputs.shape == (B, 2) and labels.shape == (B,)
    assert B % (N_CORES * P) == 0
    S = B // N_CORES
    rpp = S // P

    hb = _stage_labels_bf16(labels)
    nc, nch, plan = _build(S)
    _split_multiwaits(nc)  # idempotent; CoreSim needs the unsplit module

    import ml_dtypes

    bf16 = np.dtype(ml_dtypes.bfloat16)

    in_maps = []
    for i in range(N_CORES):
        xs = np.ascontiguousarray(outputs[i * S : (i + 1) * S], dtype=np.float32)
        xs = xs.reshape(P, 2 * rpp)
        ls = np.ascontiguousarray(hb[i * S : (i + 1) * S]).reshape(P, rpp)
        ls = ls.view(bf16)
        in_maps.append({"x": xs, "lab": ls})

    res = run_bass_kernel_spmd(nc, in_maps, core_ids=list(range(N_CORES)))
    acc = np.stack([r["acc"] for r in res.results])
    acc_h = np.stack([r["acc_h"] for r in res.results])
    return _combine(acc, acc_h, nch, plan, B), res


def kernel(outputs: np.ndarray, labels: np.ndarray) -> np.ndarray:
    return run(outputs, labels)[0]


# revision 54
# speedup vs baseline: 1.0794x; 1.0794x over previous
"""Trainium2 Bass kernel for nn_DetectionLoss (2-class detection loss).

Computes, over B=2^24 rows of logits [B,2] and labels [B]:
  ce    = mean(-log_softmax(outputs)[label])
  pred  = argmax(outputs, axis=1)
  confusion counts TP/TN/FP/FN from (label, pred)
  CS    = M[pred, label] with M = [[0,1],[0,0]]  -> mean(CS) = FN/B
  loss  = ce + coeff(TP,TN,FP,FN) * mean(CS)

Device math (2 classes): with d = x1 - x0 and h = label - 0.5 (labels
are staged host-side in their +-0.5 encoding as bf16, which is exact
for a binary label and halves their HBM footprint vs int32):
  u       = d*h                  # sign-folded logit margin
  ce_row  = softplus(-2u) = log(1 + exp(-2u))
  pred    = (d > 0)
  correct = (u > 0)              # prediction == label
Counts follow from three linear sums (n1 = sum(h) + B/2, p1 = sum(pred),
TP + TN = sum(correct) = C):
  TP = (C + p1 + n1 - B) / 2, TN = C - TP, FP = p1 - TP, FN = n1 - TP.

Engine split per chunk, balanced against the ~6.2 us the two input
DMAs of a 2048-row chunk take:
  DVE: d = x1 - x0;  u = d*h;  pred = (d>0) as fp8;
       correct = (u>0) as fp8 on the first half of columns
  ACT: t = exp(-2u); ln(1+t) with accum -> CE partial;
       Sign(u) on the second half of columns with accum
  PE : sum(h) via bf16 ones-matmuls; sum(pred)/sum(correct) via fp8
       DoubleRow ones-matmuls (2 input columns paired per output
       column - exact for 0/1 sums), accumulated in PSUM with two
       alternating banks per quantity
The tiny per-core partials are combined on the host; count arithmetic
is exact (half-integers in fp32 at every stage).

Sharding: data-parallel over the batch dim across 8 NeuronCores.
"""

import numpy as np

import concourse.bass as bass
import concourse.mybir as mybir
import concourse.tile as tile
from concourse.bass_utils import run_bass_kernel_spmd

N_CORES = 8
P = 128
LAMBD = 0.5
MMN = 512  # matmul rhs free-dim tile (one PSUM bank of fp32)

# fp8 DoubleRow matmul: pairs adjacent rhs columns, output is half-width.
PERF_MODE = mybir.MatmulPerfMode.DoubleRow

_cache = {}

_MAX_WAITS = 1  # this walrus build rejects >1 embedded sync-wait per instruction


def _split_multiwaits(nc):
    """Walrus in this container can't encode instructions with multiple
    sync waits; hoist all but the last into standalone EventSemaphore
    waits on the same engine immediately before the instruction."""
    n = [0]

    def fix_block(blk):
        new_insts = []
        for ins in blk.instructions:
            si = ins.sync_info
            if si is not None and si.on_wait and len(si.on_wait) > _MAX_WAITS:
                waits = list(si.on_wait)
                for w in waits[: -_MAX_WAITS]:
                    n[0] += 1
                    ev = mybir.InstEventSemaphore(
                        name=f"I-waitsplit-{n[0]}",
                        ins=[],
                        outs=[],
                        sync_info=mybir.SyncInfo(on_wait=[w], on_update=[]),
                    )
                    ev.engine = ins.engine
                    new_insts.append(ev)
                si.on_wait = waits[-_MAX_WAITS:]
            new_insts.append(ins)
        blk.instructions = new_insts

    for fn in nc.m.functions:
        for blk in fn.blocks:
            fix_block(blk)


def _chunk_plan(rpp: int):
    """Rows-per-partition per chunk. Small chunks at both ends shorten the
    pipeline fill (first compute can't start before chunk 0 lands) and the
    tail (last chunk's compute latency after the final DMA byte). The
    first two chunks must be >= 512 so the PSUM start slabs span their
    banks' full written width (see the matmul start= logic)."""
    if rpp == 16384:
        plan = (
            [1024, 1024, 1536]
            + [2048] * 5
            + [1024, 512, 512, 256, 256]
        )
    else:
        # small test sizes: 1024-row chunks
        assert rpp % 1024 == 0 and rpp >= 2048
        plan = [1024] * (rpp // 1024)
    assert sum(plan) == rpp and all(f % 256 == 0 for f in plan)
    assert plan[0] >= 1024 and plan[1] >= 1024
    return plan


def _eact_split(F: int) -> int:
    """Columns [0, fd) of `correct` go to DVE+PE; [fd, F) to ACT Sign.
    F <= 512 stays fully on DVE so the PSUM start slabs are full-width."""
    if F <= 1024:
        return F
    return max(1024, (5 * F // 8 // 256) * 256)


def _build(rows_per_core: int):
    """Build the per-core Bass module. All cores run the same program on
    their own shard (pure data parallel, no collectives)."""
    key = rows_per_core
    if key in _cache:
        return _cache[key]

    assert rows_per_core % P == 0
    rpp = rows_per_core // P  # rows per partition
    plan = _chunk_plan(rpp)
    nch = len(plan)
    fmax = max(plan)

    nc = bass.Bass(trn_type="TRN2")
    dtf = mybir.dt.float32
    dtb = mybir.dt.bfloat16
    dt8 = mybir.dt.float8e4
    Op = mybir.AluOpType
    Act = mybir.ActivationFunctionType

    x = nc.dram_tensor("x", [P, 2 * rpp], dtf, kind="ExternalInput")
    lab = nc.dram_tensor("lab", [P, rpp], dtb, kind="ExternalInput")
    # accumulator columns: [ce quarters | sign(u) upper part] x nch
    acc = nc.dram_tensor("acc", [P, 5 * nch], dtf, kind="ExternalOutput")
    # PE-reduced count partials: [sum(h) 2x512 | sum(pred) 2x256 |
    # sum(correct lower) 2x256]
    acc_h = nc.dram_tensor("acc_h", [1, 6 * MMN], dtf,
                           kind="ExternalOutput")

    slabs = lambda F: (F + MMN - 1) // MMN
    slabs2 = lambda F: (F + 2 * MMN - 1) // (2 * MMN)
    tot_h = sum(slabs(F) for F in plan)
    tot_p = sum(slabs2(F) for F in plan)
    tot_e = sum(slabs2(_eact_split(F)) for F in plan)

    with tile.TileContext(nc) as tc:
        with (
            tc.tile_pool(name="io", bufs=5) as io_pool,
            tc.tile_pool(name="mid", bufs=4) as mid,
            tc.tile_pool(name="junk", bufs=2) as junk,
            tc.tile_pool(name="singles", bufs=1) as singles,
            tc.tile_pool(name="ps", bufs=1, space="PSUM") as psp,
        ):
            ones = singles.tile([P, 1], dtb)
            nc.vector.memset(ones, 1.0)
            ones8 = singles.tile([P, 32], dt8)
            nc.vector.memset(ones8, 1.0)
            ones8_2 = ones8.rearrange("p (two m) -> p two m", two=2)
            st_ce = singles.tile([P, 4 * nch], dtf)
            st_sg = singles.tile([P, nch], dtf)
            nc.gpsimd.memset(st_ce, 0.0)
            nc.gpsimd.memset(st_sg, 0.0)
            ps_h = [
                psp.tile([1, MMN], dtf, tag=f"ps_h{i}", name=f"ps_h{i}")
                for i in range(2)
            ]
            ps_p = [
                psp.tile([16, MMN], dtf, tag=f"ps_p{i}", name=f"ps_p{i}")
                for i in range(2)
            ]
            ps_e = [
                psp.tile([16, MMN], dtf, tag=f"ps_e{i}", name=f"ps_e{i}")
                for i in range(2)
            ]

            r0 = 0
            ks_h = ks_p = ks_e = 0
            for c, F in enumerate(plan):
                r1 = r0 + F
                fd = _eact_split(F)
                xt_full = io_pool.tile([P, 2 * fmax], dtf, tag="xt")
                xt = xt_full[:, : 2 * F]
                nc.sync.dma_start(out=xt, in_=x[:, 2 * r0 : 2 * r1])
                xp = xt.rearrange("p (f two) -> p f two", two=2)
                lt_full = io_pool.tile([P, fmax], dtb, tag="lt")
                lv = lt_full[:, :F]
                nc.sync.dma_start(out=lv, in_=lab[:, r0:r1])

                # d = x1 - x0 and u = d*h, in column halves so the ACT
                # chain (which consumes u) can start ~half a chunk earlier
                d_full = mid.tile([P, fmax], dtb, tag="d")
                d = d_full[:, :F]
                u_full = mid.tile([P, fmax], dtb, tag="u")
                u = u_full[:, :F]
                nq0 = 4 if F >= 2048 else (2 if F >= 512 else 1)
                qw0 = (F // nq0 // 256) * 256
                cuts0 = [i * qw0 for i in range(nq0)] + [F]
                for qi in range(nq0):
                    hs = slice(cuts0[qi], cuts0[qi + 1])
                    nc.vector.tensor_sub(
                        out=d[:, hs], in0=xp[:, hs, 1], in1=xp[:, hs, 0]
                    )
                    nc.vector.tensor_mul(
                        out=u[:, hs], in0=d[:, hs], in1=lv[:, hs]
                    )
                # pred = (d > 0) as fp8 0/1
                jp_full = mid.tile([P, fmax], dt8, tag="jp")
                jp = jp_full[:, :F]
                nc.vector.tensor_scalar(
                    out=jp, in0=d, scalar1=0.0, scalar2=None, op0=Op.is_gt
                )
                # correct = (u > 0) as fp8 0/1 on columns [0, fd)
                je_full = mid.tile([P, fmax], dt8, tag="je")
                je = je_full[:, :fd]
                nc.vector.tensor_scalar(
                    out=je, in0=u[:, :fd], scalar1=0.0, scalar2=None,
                    op0=Op.is_gt
                )

                # CE partial on ACT: t = exp(-2u); ln(1+t), accum.
                # Done in two column halves so the ACT stream advances in
                # smaller steps (earlier cross-engine progress signals).
                t_full = mid.tile([P, fmax], dtb, tag="t")
                ja_full = junk.tile([P, fmax], dtb, tag="ja")
                nq = 4 if F >= 2048 else (2 if F >= 512 else 1)
                qw = (F // nq // 256) * 256
                cuts = [i * qw for i in range(nq)] + [F]
                for hi in range(nq):
                    hs = slice(cuts[hi], cuts[hi + 1])
                    th = t_full[:, hs]
                    nc.scalar.activation(
                        out=th, in_=u[:, hs], func=Act.Exp, scale=-2.0
                    )
                    jah = ja_full[:, hs]
                    nc.scalar.activation(
                        out=jah, in_=th, func=Act.Ln, bias=1.0, scale=1.0,
                        accum_out=st_ce[:, 4 * c + hi : 4 * c + hi + 1],
                    )
                # sum(sign(u)) over columns [fd, F) on ACT
                if fd < F:
                    js_full = junk.tile([P, fmax], dtb, tag="js")
                    js = js_full[:, : F - fd]
                    nc.scalar.activation(
                        out=js, in_=u[:, fd:], func=Act.Sign,
                        accum_out=st_sg[:, c : c + 1],
                    )

                # count sums on PE; banks alternate per 512-slab.
                for k in range(slabs(F)):
                    sl = slice(k * MMN, min((k + 1) * MMN, F))
                    w = sl.stop - sl.start
                    nc.tensor.matmul(
                        ps_h[ks_h % 2][:, :w], ones, lv[:, sl],
                        start=ks_h < 2, stop=ks_h >= tot_h - 2,
                    )
                    ks_h += 1
                for k in range(slabs2(F)):
                    sl = slice(k * 2 * MMN, min((k + 1) * 2 * MMN, F))
                    w = sl.stop - sl.start
                    jp2 = jp[:, sl].rearrange("p (n two) -> p two n", two=2)
                    nc.tensor.matmul(
                        ps_p[ks_p % 2][:, : w // 2], ones8_2, jp2,
                        start=ks_p < 2, stop=ks_p >= tot_p - 2,
                        perf_mode=PERF_MODE,
                    )
                    ks_p += 1
                for k in range(slabs2(fd)):
                    sl = slice(k * 2 * MMN, min((k + 1) * 2 * MMN, fd))
                    w = sl.stop - sl.start
                    je2 = je[:, sl].rearrange("p (n two) -> p two n", two=2)
                    nc.tensor.matmul(
                        ps_e[ks_e % 2][:, : w // 2], ones8_2, je2,
                        start=ks_e < 2, stop=ks_e >= tot_e - 2,
                        perf_mode=PERF_MODE,
                    )
                    ks_e += 1
                r0 = r1

            nc.sync.dma_start(out=acc[:, : 4 * nch], in_=st_ce)
            nc.sync.dma_start(out=acc[:, 4 * nch :], in_=st_sg)
            cnt_sb = singles.tile([1, 6 * MMN], dtf)
            off = 0
            for ps in ps_h + ps_p + ps_e:
                wps = ps.shape[1]
                nc.vector.tensor_copy(
                    out=cnt_sb[:, off : off + wps], in_=ps[0:1, :]
                )
                off += wps
            nc.sync.dma_start(out=acc_h[:], in_=cnt_sb)

    _cache[key] = (nc, nch, plan)
    return nc, nch, plan


def _combine(
    acc: np.ndarray, acc_h: np.ndarray, nch: int, plan, B: int
) -> np.ndarray:
    """Host-side scalar epilogue.

    acc: [n_cores, P, 2*nch] f32: [ce | sign(u) upper-column part].
    acc_h: [n_cores, 1, 2*MMN+4*256] f32 PE-reduced [sum(h) | sum(pred) |
    sum(correct lower)]. Counts are exact (half-)integers in fp32."""
    n_cores = acc.shape[0]
    a = acc.astype(np.float64)
    CE = a[:, :, : 4 * nch].sum()
    S_u = a[:, :, 4 * nch :].sum()
    ah = acc_h.astype(np.float64).reshape(n_cores, -1)
    H1 = ah[:, : 2 * MMN].sum()
    p1 = ah[:, 2 * MMN : 4 * MMN].sum()
    C_low = ah[:, 4 * MMN :].sum()
    n1 = H1 + B / 2.0  # labels == 1
    # rows covered by the ACT Sign path
    n_sign = n_cores * P * sum(F - _eact_split(F) for F in plan)
    C = C_low + (S_u + n_sign) / 2.0
    TP = (C + p1 + n1 - B) / 2.0
    TN = C - TP
    FP = p1 - TP
    FN = n1 - TP

    ce = CE / B
    mean_cs = FN / B
    nonzero = (TP > 0) and (TN > 0) and (FP > 0) and (FN > 0)
    ratio = (TP / max(TP + FN, 1.0)) * (FP / max(FP + TN, 1.0))
    if nonzero:
        coeff = -LAMBD * np.log(np.sqrt(max(ratio, 1e-30)))
    else:
        coeff = LAMBD
    return np.array(ce + coeff * mean_cs, dtype=np.float32)


def _stage_labels_bf16(labels: np.ndarray) -> np.ndarray:
    """Encode binary labels as bf16 h = label - 0.5 (+-0.5), exactly.
    0.5 -> 0x3F00, -0.5 -> 0xBF00 (as uint16 bit patterns)."""
    lab = labels.astype(bool)
    return np.where(lab, np.uint16(0x3F00), np.uint16(0xBF00))


def run(outputs: np.ndarray, labels: np.ndarray):
    """Run on 8 cores; returns (loss, BassKernelResults)."""
    outputs = np.asarray(outputs)
    labels = np.asarray(labels)
    B = outputs.shape[0]
    assert outputs.shape == (B, 2) and labels.shape == (B,)
    assert B % (N_CORES * P) == 0
    S = B // N_CORES
    rpp = S // P

    hb = _stage_labels_bf16(labels)
    nc, nch, plan = _build(S)
    _split_multiwaits(nc)  # idempotent; CoreSim needs the unsplit module

    import ml_dtypes

    bf16 = np.dtype(ml_dtypes.bfloat16)

    in_maps = []
    for i in range(N_CORES):
        xs = np.ascontiguousarray(outputs[i * S : (i + 1) * S], dtype=np.float32)
        xs = xs.reshape(P, 2 * rpp)
        ls = np.ascontiguousarray(hb[i * S : (i + 1) * S]).reshape(P, rpp)
        ls = ls.view(bf16)
        in_maps.append({"x": xs, "lab": ls})

    res = run_bass_kernel_spmd(nc, in_maps, core_ids=list(range(N_CORES)))
    acc = np.stack([r["acc"] for r in res.results])
    acc_h = np.stack([r["acc_h"] for r in res.results])
    return _combine(acc, acc_h, nch, plan, B), res


def kernel(outputs: np.ndarray, labels: np.ndarray) -> np.ndarray:
    return run(outputs, labels)[0]
